# revision 1
# baseline (speedup 1.0000x reference)
"""Trainium2 Bass kernel for the CAM (channel attention module) problem.

Computation (per batch b):
    A = inputs[b] reshaped [N=4096, C=512]
    G = A^T A                       (channel Gram matrix, [C, C])
    attn = softmax(G, axis=-1)
    out[b] = gamma * (A @ attn^T) + A

Distribution: pure data-parallel over the batch dim: 16 batches over 8
NeuronCores = 2 batches/core. No collectives.

Per-core dataflow (per batch):
  - DMA A in as 4 groups of [128, 8, 512] fp32 tiles.
  - Gram matmul directly from fp32 tiles using float32r (1 cycle/row).
  - Row max via DVE reduce (negated), S = exp(G - m) on ACT with accum_out
    giving the row sums for free.
  - T_w[j, i] = exp(G[j, i] - m_i) / s_i computed with a free-axis broadcast:
    m and 1/s are transposed to row vectors with tiny PE transposes, then
    replicated across partitions with a broadcast DMA; the shift is a DVE
    tensor_tensor add reading G straight from PSUM, exp on ACT, scale on DVE.
    (G is symmetric so bank j holds both row j and column j.)
  - A is cast to bf16 (regrouped by channel-block) and transposed with the
    DMA xbar transpose to produce A^T tiles for the second matmul's
    stationary operand.
  - Second matmul: psum = A @ T_w  (bf16).
  - Residual: out = psum * gamma + A_fp32 in one DVE scalar_tensor_tensor.
    With gamma == 0 this makes the output bit-exact equal to the input.
"""

import sys

if "/opt/trn_rl_repo" not in sys.path:
    sys.path.insert(0, "/opt/trn_rl_repo")

import numpy as np

B, H, W, C = 16, 64, 64, 512
N = H * W                 # 4096
NCORES = 8
BPC = B // NCORES         # batches per core = 2
P = 128                   # partitions
NT = N // P               # 32 n-tiles
CT = C // P               # 4 channel tiles
NGRP = 4                  # n-tile groups per batch
GNT = NT // NGRP          # 8 n-tiles per group

_BUILD_CACHE = {}


def _ml_bf16():
    import ml_dtypes

    return np.dtype(ml_dtypes.bfloat16)


def build_bass(gamma_val: float):
    import concourse.bass as bass
    import concourse.bacc as bacc
    import concourse.tile as tile
    from concourse import mybir
    from contextlib import ExitStack

    f32 = mybir.dt.float32
    f32r = mybir.dt.float32r
    bf16 = mybir.dt.bfloat16
    Exp = mybir.ActivationFunctionType.Exp
    Alu = mybir.AluOpType
    AX = mybir.AxisListType

    nc = bacc.Bacc("TRN2", target_bir_lowering=False)
    x = nc.dram_tensor("x", [BPC, N, C], f32, kind="ExternalInput")
    ident = nc.dram_tensor("ident", [P, P], f32, kind="ExternalInput")
    ones_f = nc.dram_tensor("ones_f", [1, P], f32, kind="ExternalInput")
    ones_h = nc.dram_tensor("ones_h", [1, P], bf16, kind="ExternalInput")
    y = nc.dram_tensor("y", [BPC, N, C], f32, kind="ExternalOutput")

    with tile.TileContext(nc) as tc, ExitStack() as ctx:
        singles = ctx.enter_context(tc.tile_pool(name="singles", bufs=1))
        pA = ctx.enter_context(tc.tile_pool(name="pA", bufs=5))
        pAbf = ctx.enter_context(tc.tile_pool(name="pAbf", bufs=1))
        pAT = ctx.enter_context(tc.tile_pool(name="pAT", bufs=1))
        pSm = ctx.enter_context(tc.tile_pool(name="pSm", bufs=2))
        pTmp = ctx.enter_context(tc.tile_pool(name="pTmp", bufs=2))
        pTw = ctx.enter_context(tc.tile_pool(name="pTw", bufs=6))
        pOut = ctx.enter_context(tc.tile_pool(name="pOut", bufs=3))
        pG = ctx.enter_context(tc.tile_pool(name="pG", bufs=4, space="PSUM"))
        pPo = ctx.enter_context(tc.tile_pool(name="pPo", bufs=3, space="PSUM"))
        pPv = ctx.enter_context(tc.tile_pool(name="pPv", bufs=1, space="PSUM"))

        sb_ident = singles.tile([P, P], f32)
        nc.gpsimd.dma_start(out=sb_ident, in_=ident[:, :])
        sb_ones_f = singles.tile([1, P], f32)
        nc.gpsimd.dma_start(out=sb_ones_f, in_=ones_f[:, :])
        sb_ones_h = singles.tile([1, P], bf16)
        nc.gpsimd.dma_start(out=sb_ones_h, in_=ones_h[:, :])

        for b in range(BPC):
            # ---- load A (fp32) --------------------------------------------
            Agrp = []
            for g in range(NGRP):
                Ag = pA.tile([P, GNT, C], f32, name=f"A_b{b}g{g}", tag="A")
                src = x[b, g * GNT * P:(g + 1) * GNT * P, :].rearrange(
                    "(nt p) c -> p nt c", p=P
                )
                if g == 0:
                    # split the first group so casts/gram start sooner
                    half = GNT // 2
                    nc.sync.dma_start(out=Ag[:, :half, :], in_=src[:, :half, :])
                    nc.sync.dma_start(out=Ag[:, half:, :], in_=src[:, half:, :])
                else:
                    nc.sync.dma_start(out=Ag, in_=src)
                Agrp.append(Ag)

            # ---- cast A to bf16, grouped by channel-block -----------------
            # Abf[p, jt, nt*128 + c] = bf16(A[nt*128 + p, jt*128 + c])
            Abf = pAbf.tile([P, CT, N], bf16, name=f"Abf_b{b}", tag="Abf")
            for jt in range(CT):
                for g in range(NGRP):
                    dst = Abf[:, jt, g * GNT * P:(g + 1) * GNT * P].rearrange(
                        "p (k c) -> p k c", c=P
                    )
                    src = Agrp[g][:, :, jt * P:(jt + 1) * P]
                    if jt % 2 == 0:
                        nc.vector.tensor_copy(out=dst, in_=src)
                    else:
                        nc.scalar.copy(out=dst, in_=src)

            # ---- Gram matmul (upper-triangle blocks only; G is symmetric) --
            # bank it accumulates G[it*128:(it+1)*128, it*128:512]
            G = [pG.tile([P, C], f32, name=f"G_b{b}t{it}", tag="G") for it in range(CT)]
            for g in range(NGRP):
                for k in range(GNT):
                    nt = g * GNT + k
                    for ci in range(CT):
                        nc.tensor.matmul(
                            G[ci][:, ci * P:],
                            lhsT=Abf[:, ci, nt * P:(nt + 1) * P],
                            rhs=Abf[:, ci:, nt * P:(nt + 1) * P],
                            start=(nt == 0),
                            stop=(nt == NT - 1),
                        )
            # reconstruct lower-triangle blocks: G[it][:, jt] = G[jt][:, it].T
            for it in range(1, CT):
                for jt in range(it):
                    blk = pSm.tile([P, P], f32, name=f"blk_b{b}_{it}_{jt}", tag="blk",
                                   bufs=3)
                    nc.scalar.copy(out=blk, in_=G[jt][:, it * P:(it + 1) * P])
                    nc.tensor.transpose(
                        out=G[it][:, jt * P:(jt + 1) * P],
                        in_=blk,
                        identity=sb_ident,
                    )

            # ---- DMA xbar transpose per channel block ---------------------
            # AT[c, jt*N + nt*128 + q] = bf16(A[nt*128 + q, jt*128 + c])
            AT = pAT.tile([P, CT * N], bf16, name=f"AT_b{b}", tag="AT")
            for jt in range(CT):
                nc.sync.dma_start_transpose(
                    out=AT[:, jt * N:(jt + 1) * N].rearrange(
                        "c (nt q) -> c nt q", q=P
                    ),
                    in_=Abf[:, jt, :],
                )

            # ---- softmax statistics ---------------------------------------
            negm = pSm.tile([P, CT], f32, name=f"negm_b{b}", tag="negm")
            for it in range(CT):
                nc.vector.tensor_reduce(
                    out=negm[:, it:it + 1],
                    in_=G[it],
                    axis=AX.X,
                    op=Alu.max,
                    negate=True,
                )
            s_acc = pSm.tile([P, CT], f32, name=f"s_b{b}", tag="s")
            for it in range(CT):
                S = pTmp.tile([P, C], bf16, name=f"S_b{b}t{it}", tag="S")
                nc.scalar.activation(
                    out=S,
                    in_=G[it],
                    func=Exp,
                    bias=negm[:, it:it + 1],
                    scale=1.0,
                    accum_out=s_acc[:, it:it + 1],
                )
            wrec = pSm.tile([P, CT], f32, name=f"w_b{b}", tag="w")
            nc.vector.reciprocal(out=wrec, in_=s_acc)

            # ---- transpose negm, w to row vectors; broadcast to all parts -
            vps = pPv.tile([1, C], f32, name=f"vps_b{b}", tag="vps")
            for it in range(CT):
                nc.tensor.transpose(
                    out=vps[0:1, it * P:(it + 1) * P],
                    in_=negm[:, it:it + 1],
                    identity=sb_ident,
                )
            negm_row = pSm.tile([1, C], f32, name=f"negmrow_b{b}", tag="negm_row")
            nc.scalar.copy(out=negm_row, in_=vps)

            wps = pPv.tile([1, C], f32, name=f"wps_b{b}", tag="vps")
            for it in range(CT):
                nc.tensor.transpose(
                    out=wps[0:1, it * P:(it + 1) * P],
                    in_=wrec[:, it:it + 1],
                    identity=sb_ident,
                )
            w_row = pSm.tile([1, C], bf16, name=f"wrow_b{b}", tag="w_row")
            nc.scalar.copy(out=w_row, in_=wps)

            # replicate the row vectors across partitions with rank-1 matmuls
            # (fp32 for -m so S and T use bit-identical shifts; bf16 for w)
            mrep_ps = pPv.tile([P, C], f32, name=f"mrepps_b{b}", tag="vps")
            nc.tensor.matmul(mrep_ps, lhsT=sb_ones_f, rhs=negm_row,
                             start=True, stop=True)
            NegM_rep = pSm.tile([P, C], f32, name=f"negmrep_b{b}", tag="NegM")
            nc.scalar.copy(out=NegM_rep, in_=mrep_ps)

            wrep_ps = pPv.tile([P, C], f32, name=f"wrepps_b{b}", tag="vps")
            nc.tensor.matmul(wrep_ps, lhsT=sb_ones_h, rhs=w_row,
                             start=True, stop=True)
            W_rep = pSm.tile([P, C], bf16, name=f"wrep_b{b}", tag="Wrep")
            nc.scalar.copy(out=W_rep, in_=wrep_ps)

            # ---- T_w[j, i] = exp(G[j, i] - m_i) * w_i ---------------------
            Tw = []
            for jt in range(CT):
                tmp = pTmp.tile([P, C], f32, name=f"tmp_b{b}j{jt}", tag="tmp")
                nc.vector.tensor_tensor(
                    out=tmp, in0=G[jt], in1=NegM_rep, op=Alu.add
                )
                Texp = pTmp.tile([P, C], bf16, name=f"Texp_b{b}j{jt}", tag="Texp")
                nc.scalar.activation(out=Texp, in_=tmp, func=Exp)
                Twj = pTw.tile([P, C], bf16, name=f"Tw_b{b}j{jt}", tag="Tw")
                nc.vector.tensor_mul(out=Twj, in0=Texp, in1=W_rep)
                Tw.append(Twj)

            # ---- second matmul + residual + store -------------------------
            OG = 4  # n-tiles per output store group
            for og in range(NT // OG):
                outg = pOut.tile([P, OG, C], f32, name=f"out_b{b}g{og}", tag="out")
                for k in range(OG):
                    nt = og * OG + k
                    g, kk = nt // GNT, nt % GNT
                    po = pPo.tile([P, C], f32, name=f"po_b{b}n{nt}", tag="po")
                    for jt in range(CT):
                        nc.tensor.matmul(
                            po,
                            lhsT=AT[:, jt * N + nt * P:jt * N + (nt + 1) * P],
                            rhs=Tw[jt],
                            start=(jt == 0),
                            stop=(jt == CT - 1),
                        )
                    nc.vector.scalar_tensor_tensor(
                        out=outg[:, k, :],
                        in0=po,
                        scalar=float(gamma_val),
                        in1=Agrp[g][:, kk, :],
                        op0=Alu.mult,
                        op1=Alu.add,
                    )
                nc.scalar.dma_start(
                    out=y[b, og * OG * P:(og + 1) * OG * P, :].rearrange(
                        "(nt p) c -> p nt c", p=P
                    ),
                    in_=outg,
                )
    nc.compile()
    return nc


def run(inputs_arr: np.ndarray, gamma_val: float, trace: bool = False):
    """Compile + run on the 8 cores. Returns (output [16,4096,512], results)."""
    from concourse.bass_utils import run_bass_kernel_spmd

    key = round(float(gamma_val), 12)
    if key not in _BUILD_CACHE:
        _BUILD_CACHE[key] = build_bass(float(gamma_val))
    nc = _BUILD_CACHE[key]

    xs = np.ascontiguousarray(
        np.asarray(inputs_arr, dtype=np.float32).reshape(B, N, C)
    )
    eye = np.eye(P, dtype=np.float32)
    ones_f = np.ones((1, P), dtype=np.float32)
    ones_h = np.ones((1, P), dtype=np.float32).astype(
        np.dtype("bfloat16") if hasattr(np, "bfloat16") else _ml_bf16()
    )
    in_maps = [
        {
            "x": xs[c * BPC:(c + 1) * BPC],
            "ident": eye,
            "ones_f": ones_f,
            "ones_h": ones_h,
        }
        for c in range(NCORES)
    ]
    res = run_bass_kernel_spmd(nc, in_maps, list(range(NCORES)), trace=trace)
    out = np.concatenate([res.results[c]["y"] for c in range(NCORES)], axis=0)
    return out.reshape(B, H, W, C), res


def kernel(inputs: np.ndarray, gamma: np.ndarray) -> np.ndarray:
    gamma_val = float(np.asarray(gamma).reshape(-1)[0])
    out, _ = run(inputs, gamma_val, trace=False)
    return out.astype(np.float32)


if __name__ == "__main__":
    rng = np.random.default_rng(0)
    inp = rng.standard_normal((B, H, W, C), dtype=np.float32)
    gam = np.zeros((1,), dtype=np.float32)
    out = kernel(inp, gam)
    print("shape", out.shape, "dtype", out.dtype)
    print("max|out - inp| =", np.abs(out - inp).max())



# revision 3
# speedup vs baseline: 1.0894x; 1.0894x over previous
"""Trainium2 Bass kernel for the CAM (channel attention module) problem.

Computation (per batch b):
    A = inputs[b] reshaped [N=4096, C=512]
    G = A^T A                       (channel Gram matrix, [C, C])
    attn = softmax(G, axis=-1)
    out[b] = gamma * (A @ attn^T) + A

Distribution: pure data-parallel over the batch dim: 16 batches over 8
NeuronCores = 2 batches/core. No collectives.

v2 design (vs the fp32 baseline):
  - bf16 end-to-end I/O: host casts x to bf16 (tolerance is 2e-2; bf16
    adds ~3e-3 rel-max), kernel reads/writes bf16 -> HBM traffic halved.
  - A is loaded natural [p, nt, c] into a small staging pool, then
    regrouped by channel-block (DVE/ACT copies) into Ach [p, jt, N];
    Gram matmuls and the residual read Ach, the A^T for the second
    matmul comes from an SBUF->SBUF xbar DMA transpose of Ach rows.
  - Gram accumulates upper-triangle blocks only (G symmetric); G rows
    are immediately copied PSUM->SBUF (Gs) so the 4 PSUM banks recycle
    ~1us after each Gram finishes -> the two batches pipeline on PE
    with no PSUM stall (pG=4, pPv=1, pPo=3 banks).
  - Lower-triangle blocks reconstructed into Gs by PE transposes; the
    softmax row max is taken over the stored upper row segment only
    (includes the dominant diagonal, so it is a safe max-shift).
  - Softmax-statistic PE ops (block transposes, stat transposes,
    rank-1 broadcast matmuls) are interleaved into the NEXT batch's
    matmul stream so they hide behind real PE work.
  - Residual out = gamma*po + A computed straight to bf16, stored bf16.
"""

import sys

if "/opt/trn_rl_repo" not in sys.path:
    sys.path.insert(0, "/opt/trn_rl_repo")

import numpy as np

B, H, W, C = 16, 64, 64, 512
N = H * W                 # 4096
NCORES = 8
BPC = B // NCORES         # batches per core = 2
P = 128                   # partitions
NT = N // P               # 32 n-tiles
CT = C // P               # 4 channel tiles
NGRP = 4                  # n-tile groups per batch
GNT = NT // NGRP          # 8 n-tiles per group
OG = 4                    # n-tiles per output store group

_BUILD_CACHE = {}


def _ml_bf16():
    import ml_dtypes

    return np.dtype(ml_dtypes.bfloat16)


def build_bass(gamma_val: float):
    import concourse.bass as bass
    import concourse.bacc as bacc
    import concourse.tile as tile
    from concourse import mybir
    from contextlib import ExitStack

    f32 = mybir.dt.float32
    bf16 = mybir.dt.bfloat16
    Exp = mybir.ActivationFunctionType.Exp
    Alu = mybir.AluOpType
    AX = mybir.AxisListType

    nc = bacc.Bacc("TRN2", target_bir_lowering=False)
    x = nc.dram_tensor("x", [BPC, N, C], bf16, kind="ExternalInput")
    ident = nc.dram_tensor("ident", [P, P], f32, kind="ExternalInput")
    ones_f = nc.dram_tensor("ones_f", [1, P], f32, kind="ExternalInput")
    ones_h = nc.dram_tensor("ones_h", [1, P], bf16, kind="ExternalInput")
    y = nc.dram_tensor("y", [BPC, N, C], bf16, kind="ExternalOutput")

    with tile.TileContext(nc) as tc, ExitStack() as ctx:
        singles = ctx.enter_context(tc.tile_pool(name="singles", bufs=1))
        pStg = ctx.enter_context(tc.tile_pool(name="pStg", bufs=2))
        pAch = ctx.enter_context(tc.tile_pool(name="pAch", bufs=2))
        pAT = ctx.enter_context(tc.tile_pool(name="pAT", bufs=2))
        pGs = ctx.enter_context(tc.tile_pool(name="pGs", bufs=2))
        pSm = ctx.enter_context(tc.tile_pool(name="pSm", bufs=2))
        pTmp = ctx.enter_context(tc.tile_pool(name="pTmp", bufs=2))
        pTw = ctx.enter_context(tc.tile_pool(name="pTw", bufs=8))
        pOut = ctx.enter_context(tc.tile_pool(name="pOut", bufs=3))
        pG = ctx.enter_context(tc.tile_pool(name="pG", bufs=4, space="PSUM"))
        pPv = ctx.enter_context(tc.tile_pool(name="pPv", bufs=1, space="PSUM"))
        pPo = ctx.enter_context(tc.tile_pool(name="pPo", bufs=3, space="PSUM"))

        sb_ident = singles.tile([P, P], f32)
        nc.gpsimd.dma_start(out=sb_ident, in_=ident[:, :])
        sb_ones_f = singles.tile([1, P], f32)
        nc.gpsimd.dma_start(out=sb_ones_f, in_=ones_f[:, :])
        sb_ones_h = singles.tile([1, P], bf16)
        nc.gpsimd.dma_start(out=sb_ones_h, in_=ones_h[:, :])

        st = [dict() for _ in range(BPC)]

        # ---- loads + channel-block regroup -------------------------------
        def emit_load_regroup(b):
            Ach = pAch.tile([P, CT, N], bf16, name=f"Ach_b{b}", tag="Ach")
            for g in range(NGRP):
                stg = pStg.tile([P, GNT, C], bf16, name=f"stg_b{b}g{g}", tag="stg")
                src = x[b, g * GNT * P:(g + 1) * GNT * P, :].rearrange(
                    "(nt p) c -> p nt c", p=P
                )
                if b == 0 and g == 0:
                    half = GNT // 2
                    nc.sync.dma_start(out=stg[:, :half, :], in_=src[:, :half, :])
                    nc.sync.dma_start(out=stg[:, half:, :], in_=src[:, half:, :])
                else:
                    nc.sync.dma_start(out=stg, in_=src)
                for jt in range(CT):
                    dst = Ach[:, jt, g * GNT * P:(g + 1) * GNT * P].rearrange(
                        "p (k q) -> p k q", q=P
                    )
                    srcs = stg[:, :, jt * P:(jt + 1) * P]
                    if jt % 2 == 0:
                        nc.vector.tensor_copy(out=dst, in_=srcs)
                    else:
                        nc.scalar.copy(out=dst, in_=srcs)
            st[b]["Ach"] = Ach

        # ---- A^T via SBUF->SBUF xbar transpose ---------------------------
        def emit_at(b):
            ATb = pAT.tile([P, CT, N], bf16, name=f"AT_b{b}", tag="AT")
            for jt in range(CT):
                nc.sync.dma_start_transpose(
                    out=ATb[:, jt, :].rearrange("c (nt q) -> c nt q", q=P),
                    in_=st[b]["Ach"][:, jt, :],
                )
            st[b]["AT"] = ATb

        # ---- Gram (upper-triangle blocks), with interleaved side ops -----
        def emit_gram(b, side_ops=()):
            side = list(side_ops)
            Ach = st[b]["Ach"]
            G = [
                pG.tile([P, C], f32, name=f"G_b{b}c{ci}", tag="G")
                for ci in range(CT)
            ]
            for nt in range(NT):
                for ci in range(CT):
                    nc.tensor.matmul(
                        G[ci][:, ci * P:],
                        lhsT=Ach[:, ci, nt * P:(nt + 1) * P],
                        rhs=Ach[:, ci:, nt * P:(nt + 1) * P],
                        start=(nt == 0),
                        stop=(nt == NT - 1),
                    )
                if side and nt % 2 == 1:
                    side.pop(0)()
            while side:
                side.pop(0)()
            st[b]["G"] = G

        # ---- G rows PSUM->SBUF + row-max (negated) -----------------------
        def emit_stats(b):
            G = st[b]["G"]
            Gs = pGs.tile([P, CT, C], f32, name=f"Gs_b{b}", tag="Gs")
            for ci in range(CT):
                eng = nc.vector.tensor_copy if ci % 2 == 0 else nc.scalar.copy
                eng(out=Gs[:, ci, ci * P:], in_=G[ci][:, ci * P:])
            negm = pSm.tile([P, CT], f32, name=f"negm_b{b}", tag="negm")
            for it in range(CT):
                nc.vector.tensor_reduce(
                    out=negm[:, it:it + 1],
                    in_=Gs[:, it, it * P:],
                    axis=AX.X,
                    op=Alu.max,
                    negate=True,
                )
            st[b]["Gs"] = Gs
            st[b]["negm"] = negm

        # ---- softmax tail as a list of closures (interleaved into the
        #      next batch's PE stream) -------------------------------------
        def softmax_closures(b):
            Gs = st[b]["Gs"]
            negm = st[b]["negm"]
            s_acc = pSm.tile([P, CT], f32, name=f"s_b{b}", tag="s")
            wrec = pSm.tile([P, CT], f32, name=f"w_b{b}", tag="w")
            st[b]["Tw"] = [None] * CT
            ops = []

            # reconstruct lower-triangle blocks of Gs rows
            def blk_t(it, jt, b=b, Gs=Gs):
                tr = pPv.tile([P, P], f32, name=f"tr_b{b}_{it}_{jt}", tag="pv")
                nc.tensor.transpose(
                    out=tr, in_=Gs[:, jt, it * P:(it + 1) * P], identity=sb_ident
                )
                nc.scalar.copy(out=Gs[:, it, jt * P:(jt + 1) * P], in_=tr)

            for it in range(1, CT):
                for jt in range(it):
                    ops.append(lambda it=it, jt=jt: blk_t(it, jt))

            # exp over full rows, accumulating row sums; then 1/s
            def s_pass(b=b, Gs=Gs, negm=negm, s_acc=s_acc, wrec=wrec):
                for it in range(CT):
                    S = pTmp.tile([P, C], bf16, name=f"S_b{b}t{it}", tag="S")
                    nc.scalar.activation(
                        out=S,
                        in_=Gs[:, it, :],
                        func=Exp,
                        bias=negm[:, it:it + 1],
                        scale=1.0,
                        accum_out=s_acc[:, it:it + 1],
                    )
                nc.vector.reciprocal(out=wrec, in_=s_acc)

            ops.append(s_pass)

            # transpose the [128,1] stat columns to a [1,512] row
            def col_to_row(src, row, b=b):
                vps = pPv.tile([1, C], f32, name=f"vps_{id(row)}", tag="pv")
                for it in range(CT):
                    nc.tensor.transpose(
                        out=vps[0:1, it * P:(it + 1) * P],
                        in_=src[:, it:it + 1],
                        identity=sb_ident,
                    )
                nc.scalar.copy(out=row, in_=vps)

            negm_row = pSm.tile([1, C], f32, name=f"negmrow_b{b}", tag="nrow")
            ops.append(lambda: col_to_row(negm, negm_row))
            w_row = pSm.tile([1, C], bf16, name=f"wrow_b{b}", tag="wrow")
            ops.append(lambda: col_to_row(wrec, w_row))

            # broadcast the rows across partitions with rank-1 matmuls
            NegM_rep = pSm.tile([P, C], f32, name=f"negmrep_b{b}", tag="mrep")
            W_rep = pSm.tile([P, C], bf16, name=f"wrep_b{b}", tag="wrep")

            def rank1(onesv, row, rep, b=b):
                ps = pPv.tile([P, C], f32, name=f"rep_{id(rep)}", tag="pv")
                nc.tensor.matmul(ps, lhsT=onesv, rhs=row, start=True, stop=True)
                nc.scalar.copy(out=rep, in_=ps)

            ops.append(lambda: rank1(sb_ones_f, negm_row, NegM_rep))
            ops.append(lambda: rank1(sb_ones_h, w_row, W_rep))

            # T_w[j, i] = exp(G[j, i] - m_i) * w_i  (G symmetric)
            def tw_j(jt, b=b, Gs=Gs):
                tmp = pTmp.tile([P, C], f32, name=f"tmp_b{b}j{jt}", tag="tmp")
                nc.vector.tensor_tensor(
                    out=tmp, in0=Gs[:, jt, :], in1=NegM_rep, op=Alu.add
                )
                Texp = pTmp.tile([P, C], bf16, name=f"Texp_b{b}j{jt}", tag="Texp")
                nc.scalar.activation(out=Texp, in_=tmp, func=Exp)
                Twj = pTw.tile([P, C], bf16, name=f"Tw_b{b}j{jt}", tag="Tw")
                nc.vector.tensor_mul(out=Twj, in0=Texp, in1=W_rep)
                st[b]["Tw"][jt] = Twj

            for jt in range(CT):
                ops.append(lambda jt=jt: tw_j(jt))
            return ops

        # ---- second matmul + residual + store ----------------------------
        def emit_mm2(b, side_ops=()):
            side = list(side_ops)
            Ach = st[b]["Ach"]
            ATb = st[b]["AT"]
            for og in range(NT // OG):
                outg = pOut.tile(
                    [P, OG, C], bf16, name=f"out_b{b}g{og}", tag="out"
                )
                for k in range(OG):
                    nt = og * OG + k
                    po = pPo.tile([P, C], f32, name=f"po_b{b}n{nt}", tag="po")
                    for jt in range(CT):
                        nc.tensor.matmul(
                            po,
                            lhsT=ATb[:, jt, nt * P:(nt + 1) * P],
                            rhs=st[b]["Tw"][jt],
                            start=(jt == 0),
                            stop=(jt == CT - 1),
                        )
                    nc.vector.scalar_tensor_tensor(
                        out=outg[:, k, :].rearrange("p (jt q) -> p jt q", q=P),
                        in0=po.rearrange("p (jt q) -> p jt q", q=P),
                        scalar=float(gamma_val),
                        in1=Ach[:, :, nt * P:(nt + 1) * P],
                        op0=Alu.mult,
                        op1=Alu.add,
                    )
                    if side and nt % 2 == 1:
                        side.pop(0)()
                nc.scalar.dma_start(
                    out=y[b, og * OG * P:(og + 1) * OG * P, :].rearrange(
                        "(nt p) c -> p nt c", p=P
                    ),
                    in_=outg,
                )
            while side:
                side.pop(0)()

        # ---- schedule ----------------------------------------------------
        emit_load_regroup(0)
        emit_load_regroup(1)
        emit_at(0)
        emit_at(1)
        emit_gram(0)
        emit_stats(0)
        emit_gram(1, side_ops=softmax_closures(0))
        emit_stats(1)
        emit_mm2(0, side_ops=softmax_closures(1))
        emit_mm2(1)

    nc.compile()
    return nc


def run(inputs_arr: np.ndarray, gamma_val: float, trace: bool = False):
    """Compile + run on the 8 cores. Returns (output [16,64,64,512], results)."""
    from concourse.bass_utils import run_bass_kernel_spmd

    key = round(float(gamma_val), 12)
    if key not in _BUILD_CACHE:
        _BUILD_CACHE[key] = build_bass(float(gamma_val))
    nc = _BUILD_CACHE[key]

    bf16 = _ml_bf16()
    xs = np.asarray(inputs_arr, dtype=np.float32).reshape(B, N, C).astype(bf16)
    xs = np.ascontiguousarray(xs)
    eye = np.eye(P, dtype=np.float32)
    ones_f = np.ones((1, P), dtype=np.float32)
    ones_h = np.ones((1, P), dtype=np.float32).astype(bf16)
    in_maps = [
        {
            "x": xs[c * BPC:(c + 1) * BPC],
            "ident": eye,
            "ones_f": ones_f,
            "ones_h": ones_h,
        }
        for c in range(NCORES)
    ]
    res = run_bass_kernel_spmd(nc, in_maps, list(range(NCORES)), trace=trace)
    out = np.concatenate(
        [np.asarray(res.results[c]["y"]) for c in range(NCORES)], axis=0
    )
    return out.astype(np.float32).reshape(B, H, W, C), res


def kernel(inputs: np.ndarray, gamma: np.ndarray) -> np.ndarray:
    gamma_val = float(np.asarray(gamma).reshape(-1)[0])
    out, _ = run(inputs, gamma_val, trace=False)
    return out.astype(np.float32)


if __name__ == "__main__":
    rng = np.random.default_rng(0)
    inp = rng.standard_normal((B, H, W, C), dtype=np.float32)
    gam = np.zeros((1,), dtype=np.float32)
    out = kernel(inp, gam)
    print("shape", out.shape, "dtype", out.dtype)
    print("max|out - inp| =", np.abs(out - inp).max())
